# revision 31
# baseline (speedup 1.0000x reference)
"""Trainium2 Bass kernel for nn_CP2_17669495456475 (dynamic-kernel deconv).

Math: out[b,c,y,x] = sum_l cos[b,l,i,j] * W[b,l,c,ky,kx],  y=8i+ky, x=8j+kx,
with W = unfold(pad(b)) * (1 - unfold(pad(mask))), K=16, S=8, crop 4.

Factored form (per core): with ky = ry + 8*sy, kx = rx + 8*sx and
t = li + sy, s = lj + sx, the (l, sy, sx) contraction of size 4096
collapses onto the 33x33 block grid (size 1089):

  outT[(c,ry,rx), (u,v)] = sum_{t,s} bm[(t,s), (c,ry,rx)] * Y[(t,s), u, v]
  Y[(t,s), u, v]         = sum_{sy,sx in {0,1}} Xp[(t-sy, s-sx), u+1-sy, v+1-sx]

where bm = blocked pad(b)*(1-pad(mask)) (each block used once - the unfold
duplication is gone) and Y is a cheap 4-term shifted pre-sum of the cos
planes.  The deconv overlap-add is absorbed into PSUM accumulation.
3.55x fewer matmul FLOPs than the expanded form and 3.8x less W traffic;
the kernel sits at the compute/memory ridge.

Host does layout glue + the O(N) prep (replicate pad, block reshape, mask
premultiply, the 4-term Y pre-sum, zero pad, final crop/assembly); the
device does the full 2.6 GFLOP/core GEMM.

Sharding: 8 cores = 4 batches x 2 channel-halves (16 ch each). Full inputs
in, full output out.
"""

import numpy as np

import concourse.bass as bass
import concourse.mybir as mybir
import concourse.tile as tile
from concourse.bass_utils import run_bass_kernel_spmd

PD = 4
C = 16              # channels per core
N_CORES = 8
NCHUNK = 9          # ceil(1089 / 128) contraction chunks
NV = 33
NT = 11 * NV        # N per matmul: 11 u-rows x 33 v-cols
# W chunk-group split: phase 0 is chunk-outer, so early chunks must land
# first; singles up front, bigger (better-packet) groups later.
WGROUPS = [(0, 1), (1, 2), (2, 3), (3, 4), (4, 5), (5, 7), (7, 9)]


def _split_multi_sync(nc):
    """The walrus in this env allows only ONE sync-wait per instruction.
    Hoist extra waits onto same-engine InstNoOp carriers placed just before
    the owning instruction (sequential waits on one engine == AND)."""
    ctr = 0
    for f in nc.m.functions:
        for bb in f.blocks:
            insts = list(bb.instructions)
            out = []
            changed = False
            for inst in insts:
                si = inst.sync_info
                waits = list(si.on_wait) if si and si.on_wait else []
                if len(waits) > 1:
                    for w in waits[:-1]:
                        nop = mybir.InstNoOp(name=f"waitnop-{ctr}", ins=[], outs=[])
                        ctr += 1
                        nop.engine = inst.engine
                        nop.sync_info = mybir.SyncInfo(on_wait=[w], on_update=[])
                        out.append(nop)
                    si.on_wait = [waits[-1]]
                    changed = True
                out.append(inst)
            if changed:
                bb.instructions = out
    return ctr


def _build_nc():
    f32 = mybir.dt.float32
    bf16 = mybir.dt.bfloat16
    nc = bass.Bass(enable_partition_id=False)
    # W: [p, chunk, (c,ry,rx)] bf16, 2KB/partition per chunk, mask
    # premultiplied host-side.  Row 128*chunk+p = block (t,s) = divmod(.,33);
    # rows >= 1089 zero-padded.
    w = nc.declare_dram_parameter("w", [128, NCHUNK, C * 64], bf16, isOutput=False)
    # Y pre-sum, y-slab layout.  Slab n holds u-rows 11n..11n+10 (no overlap
    # since the (sy,sx) shifts are folded into Y).  Slab 0 is chunk-granular
    # so the first matmul waits on a single 0.1MB entry.
    yp0 = nc.declare_dram_parameter("yp0", [128, NCHUNK, 11, 36], bf16, isOutput=False)
    # slab 1 split by chunk range so phase 1's first matmuls gate on the
    # smaller early entry instead of one 2.2us entry at the queue tail
    yp1a = nc.declare_dram_parameter("yp1a", [128, 11, 5, 36], bf16, isOutput=False)
    yp1b = nc.declare_dram_parameter("yp1b", [128, 11, 4, 36], bf16, isOutput=False)
    yp2 = nc.declare_dram_parameter("yp2", [128, 11, NCHUNK, 36], bf16, isOutput=False)
    # out: phase-major [n, p, m, NT] bf16 (host upcasts; halves out traffic)
    outT = nc.declare_dram_parameter("outT", [3, 128, 8, NT], bf16, isOutput=True)

    with tile.TileContext(nc) as tc:
        with (
            tc.tile_pool(name="yp", bufs=1) as ypp,
            tc.tile_pool(name="wp", bufs=1) as wp,
            tc.tile_pool(name="op", bufs=6) as op,
            tc.tile_pool(name="pp", bufs=8, space="PSUM") as pp,
        ):
            # Streaming schedule.  Every DMA engine round-robins WHOLE ring
            # entries between the two queues (entries serialize per engine;
            # descriptor fetch hides only while the other queue streams), so
            # delivery order is the entry-alternation order:
            #   Wc0, Y0g0, Wc1, Wc2, Y0g1, Wc3, Wc4, Y0g2, Wc5-6, Wc7-8,
            #   slab1, slab2 -- every entry lands >1.3us before PE demand.
            wts = [None] * len(WGROUPS)
            y0g = [None] * NCHUNK

            def wdma(eng, gi):
                c0, c1 = WGROUPS[gi]
                wt = wp.tile([128, c1 - c0, C * 64], bf16, name=f"w_{gi}")
                eng.dma_start(wt[:], w[:, c0:c1, :])
                wts[gi] = wt

            def ydma(eng, c):
                yt = ypp.tile([128, 11, 36], bf16, name=f"y0_{c}")
                eng.dma_start(yt[:], yp0[:, c])
                y0g[c] = yt

            # chunk 0's W is split across both queue HEADS so the first
            # matmul waits only on two half-entries + y0c0.
            wc0a = wp.tile([128, 512], bf16, name="wc0a")
            nc.sync.dma_start(wc0a[:], w[:, 0, 0:512])
            wc0b = wp.tile([128, 512], bf16, name="wc0b")
            nc.scalar.dma_start(wc0b[:], w[:, 0, 512:1024])
            ydma(nc.sync, 0)
            ydma(nc.scalar, 1)
            wdma(nc.sync, 1)      # chunk 1
            wdma(nc.scalar, 2)    # chunk 2
            ydma(nc.sync, 2)
            ydma(nc.scalar, 3)
            wdma(nc.sync, 3)      # chunk 3
            wdma(nc.scalar, 4)    # chunk 4
            ydma(nc.sync, 4)
            ydma(nc.scalar, 5)
            wdma(nc.sync, 5)      # chunks 5-6
            wdma(nc.scalar, 6)    # chunks 7-8
            ydma(nc.sync, 6)
            ydma(nc.scalar, 7)
            ydma(nc.sync, 8)
            s1a = ypp.tile([128, 11, 5, 36], bf16, name="y_1a")
            nc.scalar.dma_start(s1a[:], yp1a[:])
            s2 = ypp.tile([128, 11, NCHUNK, 36], bf16, name="y_2")
            nc.sync.dma_start(s2[:], yp2[:])
            s1b = ypp.tile([128, 11, 4, 36], bf16, name="y_1b")
            nc.scalar.dma_start(s1b[:], yp1b[:])

            def lhsT(c, m):
                if c == 0:
                    t = wc0a if m < 4 else wc0b
                    return t[:, 128 * (m % 4):128 * (m % 4 + 1)]
                for gi, (c0, c1) in enumerate(WGROUPS):
                    if c0 <= c < c1:
                        return wts[gi][:, c - c0, 128 * m:128 * (m + 1)]
                raise AssertionError(c)

            def rhs_ap(n, c, v0=0, v1=NV):
                if n == 0:
                    return y0g[c][:, :, v0:v1]
                if n == 1:
                    if c < 5:
                        return s1a[:, :, c, v0:v1]
                    return s1b[:, :, c - 5, v0:v1]
                return s2[:, :, c, v0:v1]

            # per-2m writeback tiles: dependency tracking is whole-tile, so
            # pair tiles let earlier output DMAs fly while later psum copies
            # are still pending.  Out entries alternate queues; the final two
            # m-groups go out as singles so the post-last-matmul drain is one
            # small entry on a fresh queue.

            # Phase 0 is chunk-outer with 8 live psum groups so the PE
            # consumes W/Y chunk DMAs as they stream.
            pss = [pp.tile([128, NT], f32, tag="ps", name=f"ps_0_{m}")
                   for m in range(8)]
            for c in range(NCHUNK):
                r = rhs_ap(0, c)
                for m in range(8):
                    nc.tensor.matmul(pss[m][:], lhsT(c, m), r,
                                     start=(c == 0), stop=(c == NCHUNK - 1))
            for k in range(4):
                ot = op.tile([128, 2, NT], bf16, tag="o", name=f"osb_0_{k}")
                nc.vector.tensor_copy(ot[:, 0, :], pss[2 * k][:])
                nc.vector.tensor_copy(ot[:, 1, :], pss[2 * k + 1][:])
                eng = nc.sync if k % 2 == 0 else nc.scalar
                eng.dma_start(outT[0, :, 2 * k:2 * k + 2], ot[:])

            # Phases 1-2 run m-outer (all data resident) so each group's
            # psum copy + output DMA overlaps the next group's matmuls.
            # Phase-2 pairs alternate queues, and the final two groups go
            # out as singles on both queues so the post-last-matmul drain
            # is one small entry per queue.
            for n in (1, 2):
                ot = None
                for m in range(8):
                    if n == 2 and m == 7:
                        # the final group runs as two half-width psum groups:
                        # the first half's copy+DMA drain while the second
                        # half's matmuls still stream.
                        for half, (u0, u1) in enumerate(((0, 6), (6, 11))):
                            nw = (u1 - u0) * NV
                            psh = pp.tile([128, nw], f32, tag="ps",
                                          name=f"ps_2_7{half}")
                            for c in range(NCHUNK):
                                nc.tensor.matmul(
                                    psh[:], lhsT(c, 7),
                                    s2[:, u0:u1, c, 0:NV],
                                    start=(c == 0), stop=(c == NCHUNK - 1))
                            oth = op.tile([128, nw], bf16, tag="o",
                                          name=f"osb_2_7{half}")
                            nc.vector.tensor_copy(oth[:], psh[:])
                            eng = nc.sync if half == 0 else nc.scalar
                            eng.dma_start(
                                outT[2, :, 7, u0 * NV:u1 * NV], oth[:])
                        continue
                    ps = pp.tile([128, NT], f32, tag="ps", name=f"ps_{n}_{m}")
                    for c in range(NCHUNK):
                        nc.tensor.matmul(ps[:], lhsT(c, m), rhs_ap(n, c),
                                         start=(c == 0), stop=(c == NCHUNK - 1))
                    if n == 2 and m >= 4:
                        # steady single-m trickle alternating queues: an idle
                        # DMA queue pays a ~2us cold wakeup, so keep both warm
                        # through the final two half-entries.
                        ot = op.tile([128, 1, NT], bf16, tag="o",
                                     name=f"osb_{n}_s{m}")
                        nc.vector.tensor_copy(ot[:, 0, :], ps[:])
                        eng = nc.scalar if m % 2 == 0 else nc.sync
                        eng.dma_start(outT[n, :, m:m + 1], ot[:])
                        continue
                    if m % 2 == 0:
                        ot = op.tile([128, 2, NT], bf16, tag="o",
                                     name=f"osb_{n}_{m // 2}")
                    nc.vector.tensor_copy(ot[:, m % 2, :], ps[:])
                    if m % 2 == 1:
                        eng = nc.sync if (n * 4 + m // 2) % 2 == 0 else nc.scalar
                        eng.dma_start(outT[n, :, m - 1:m + 1], ot[:])

    _split_multi_sync(nc)
    return nc


def _host_prep_batch(cos_b):
    """cos_b (1024,32,32) f32 -> Y dram blobs shared by both cores of the
    batch.  Y[t,s,u,v] = sum of 4 shifted cos planes on the 33x33 block
    grid."""
    X4 = cos_b.reshape(32, 32, 32, 32)
    Y = np.zeros((33, 33, 33, 36), np.float32)
    for sy in (0, 1):
        for sx in (0, 1):
            Y[sy:32 + sy, sx:32 + sx, sy:32 + sy, sx:32 + sx] += X4
    import ml_dtypes
    Yf = np.zeros((NCHUNK * 128, 33, 36), np.float32)
    Yf[:33 * 33] = Y.reshape(33 * 33, 33, 36)
    # [128c+p, u, x] -> [p, u, c, x] (slabs 1,2) and [p, g, u, c', x] (slab 0)
    Yc = Yf.reshape(NCHUNK, 128, 33, 36).astype(ml_dtypes.bfloat16)
    yp0 = np.ascontiguousarray(Yc[:, :, 0:11, :].transpose(1, 0, 2, 3))
    yp1a = np.ascontiguousarray(Yc[0:5, :, 11:22, :].transpose(1, 2, 0, 3))
    yp1b = np.ascontiguousarray(Yc[5:9, :, 11:22, :].transpose(1, 2, 0, 3))
    yp2 = np.ascontiguousarray(Yc[:, :, 22:33, :].transpose(1, 2, 0, 3))
    return {"yp0": yp0, "yp1a": yp1a, "yp1b": yp1b, "yp2": yp2}


def _host_prep_w(b_ch, mask_b):
    """b_ch (16,256,256), mask_b (256,256) f32 -> w [128, 9, 1024] bf16,
    mask premultiplied, blocked on the 33x33 grid, zero-padded to 1152."""
    bpad = np.pad(b_ch, ((0, 0), (PD, PD), (PD, PD)), mode="edge")
    mpad = np.pad(mask_b, ((PD, PD), (PD, PD)), mode="edge")
    bT = bpad.reshape(C, 33, 8, 33, 8).transpose(1, 3, 0, 2, 4).reshape(33 * 33, C, 64)
    mT = mpad.reshape(33, 8, 33, 8).transpose(0, 2, 1, 3).reshape(33 * 33, 64)
    bm = bT * (1.0 - mT)[:, None, :]
    import ml_dtypes
    wf = np.zeros((NCHUNK * 128, C * 64), ml_dtypes.bfloat16)
    wf[:33 * 33] = bm.reshape(33 * 33, C * 64)
    return np.ascontiguousarray(wf.reshape(NCHUNK, 128, C * 64).transpose(1, 0, 2))


def _unshard(outT):
    # outT [3, 128, 8, 363] -> [(c,ry,rx)=128m+p, u=11n+u', v] -> (16,256,256)
    outT = np.asarray(outT, dtype=np.float32)
    t = outT.reshape(3, 128, 8, 11, NV).transpose(2, 1, 0, 3, 4).reshape(1024, 33, NV)
    t = t.reshape(C, 8, 8, 33, 33).transpose(0, 3, 1, 4, 2)
    return t.reshape(C, 264, 264)[:, 4:260, 4:260]


_RUN_KW = {}   # test harness may inject e.g. trace=True
_LAST_RESULTS = [None]
_NC_CACHE = {}


def _get_nc():
    nc = _NC_CACHE.get("nc")
    if nc is None:
        nc = _NC_CACHE["nc"] = _build_nc()
    return nc


def kernel(cos_similar, b, mask):
    cos_similar = np.ascontiguousarray(np.asarray(cos_similar, dtype=np.float32))
    b = np.ascontiguousarray(np.asarray(b, dtype=np.float32))
    mask = np.ascontiguousarray(np.asarray(mask, dtype=np.float32))

    y_maps = [_host_prep_batch(cos_similar[batch]) for batch in range(4)]
    in_maps = []
    for core in range(N_CORES):
        batch, half = core // 2, core % 2
        ch0 = C * half
        m = dict(y_maps[batch])
        m["w"] = _host_prep_w(b[batch, ch0:ch0 + C], mask[batch, 0])
        in_maps.append(m)

    nc = _get_nc()
    res = run_bass_kernel_spmd(nc, in_maps, list(range(N_CORES)), **_RUN_KW)
    _LAST_RESULTS[0] = res

    out = np.empty((4, 32, 256, 256), np.float32)
    for core in range(N_CORES):
        batch, half = core // 2, core % 2
        ch0 = C * half
        out[batch, ch0:ch0 + C] = _unshard(res.results[core]["outT"])
    return out
